# revision 7
# baseline (speedup 1.0000x reference)
"""Trainium2 Bass kernel: 5-point Laplacian smoothness loss over an 8192x8192
float32 matrix, sharded across 8 NeuronCores.

loss = 0.1 * sum_{i,j interior} | a[i,j] - 0.25*(a[i-1,j]+a[i+1,j]+a[i,j-1]+a[i,j+1]) |
     = 0.025 * sum | 4a[i,j] - (a[i-1,j]+a[i+1,j]+a[i,j-1]+a[i,j+1]) |

Per-core plan (PE-bound at ~216 ns per 512-col bf16 matmul):
  - host casts to bf16; global interior rows 1..8190 are covered by 65 row
    tiles of 126 output rows each (128 input rows, 2-row overlap). 64 tiles
    are distributed 8-per-core via contiguous [1010, 8192] shards; the 65th
    (rows 8065..8190) is split column-wise across the cores (1022 output
    cols each via a [128, 1024] slab), plus a shared 14-col strip computed
    only with core 0's data (other cores get a zero strip). This removes
    the baseline's 9th near-empty row tile (-11% PE work).
  - per 128-row tile: vertical stencil via TensorE matmul with tridiagonal
    (1,-4,1) bf16 stationary into PSUM; horizontal l+r via one fused DVE
    shifted bf16 add per half-tile, accumulated with identity matmuls.
    Matmuls are emitted in stationary-batched quads (T,T,T,T then I,I,I,I).
  - PSUM: 8 groups of 1024 f32 cols (2 banks each), 4 group-buffers in
    flight so the abs-reduce latency never stalls the PE.
  - abs+row-sum from PSUM: ScalarE activation(Abs, accum_out) on 6 groups,
    VectorE tensor_reduce(abs) on 2 (DVE also does the h-adds; measured
    rates: ScalarE (FD+352)/1.2ns, DVE h-add (58+FD/2)/0.96ns, DVE reduce
    (120+FD)/0.96ns -> ScalarE ~56.2us, DVE ~55.5us, PE ~56.2us; measured
    HW exec ~56.0us, i.e. at the PE floor of this 2-pass bf16 design).
    fp8 DoubleRow matmuls (0.5 cyc/col) would cut PE to ~34us but any
    DoubleRow matmul that is not the first matmul of the program wedges
    the device (NRT_EXEC_UNIT_UNRECOVERABLE), and walrus codegen rejects
    ldw-opt + SwInterleave variants, so that path is unusable here.
  - output [1024, 10] f32 per core: cols 0:8 per-row-group sums of the 8
    main tiles, col 8 the tail slab, col 9 the strip; host does the final
    float64 sum * 0.025.
"""

import numpy as np

N = 8192
NCORES = 8
TILE_OUT = 126                        # output rows per main tile
TILES_PER_CORE = 8
SHARD_ROWS = TILES_PER_CORE * TILE_OUT + 2   # 1010
TAIL_ROW0 = 64 * TILE_OUT                    # 8064: input row 0 of tail tile
TAIL_W = 1024                          # tail slab input cols per core
TAIL_OUT = 1022                        # valid output cols per tail slab
STRIP_C0 = 8 * TAIL_OUT                # 8176: strip input col 0
STRIP_W = 16                           # strip input cols (outputs 8177..8190)
LAMBDA_SMOOTH = 0.1
NG = 8                                 # PSUM groups per main tile (1024 cols)

# center-column chunks (512-wide, last 510); groups of 2 chunks
_CHUNKS = [(1 + 512 * i, min(1 + 512 * (i + 1), 8191)) for i in range(16)]
_GROUPS = [(_CHUNKS[2 * g][0], _CHUNKS[2 * g + 1][1], _CHUNKS[2 * g:2 * g + 2])
           for g in range(NG)]
# abs-reduce engine per group: 'S' = ScalarE activation, 'V' = DVE reduce
_ASSIGN = "SSVSSSVS"
# h halves: half 0 covers groups 0-3 (cols 1..4097), half 1 groups 4-7
_HALVES = [(1, 4097), (4097, 8191)]

_NC_CACHE = {}


def _build_nc(reps=1, mode="full"):
    """mode: 'full' | 'dma' (loads only) | 'dve' (+h-adds) | 'pe' (+matmuls)."""
    import concourse.tile as tile
    from concourse import bacc, mybir

    f32 = mybir.dt.float32
    bf16 = mybir.dt.bfloat16
    nc = bacc.Bacc("TRN2", target_bir_lowering=False, debug=False)
    a_dram = nc.declare_dram_parameter("a", [SHARD_ROWS, N], bf16, isOutput=False)
    a2_dram = nc.declare_dram_parameter("a2", [128, TAIL_W], bf16, isOutput=False)
    a3_dram = nc.declare_dram_parameter("a3", [128, STRIP_W], bf16, isOutput=False)
    w_dram = nc.declare_dram_parameter("w", [256, 128], bf16, isOutput=False)
    out_dram = nc.declare_dram_parameter("out", [1024, 10], f32, isOutput=True)

    with tile.TileContext(nc) as tc:
        with (
            tc.tile_pool(name="wpool", bufs=1) as wpool,
            tc.tile_pool(name="apool", bufs=5) as apool,
            tc.tile_pool(name="hpool", bufs=4) as hpool,
            tc.tile_pool(name="scpool", bufs=4) as scpool,
            tc.tile_pool(name="rpool", bufs=3) as rpool,
            tc.tile_pool(name="pspool", bufs=4, space="PSUM") as pspool,
        ):
            # stationary matrices: cols 0:128 tridiag(1,-4,1), cols 128:256 identity
            w_t = wpool.tile([128, 256], bf16)
            nc.gpsimd.dma_start(out=w_t[:, 0:128], in_=w_dram[0:128, :])
            nc.gpsimd.dma_start(out=w_t[:, 128:256], in_=w_dram[128:256, :])
            s4 = w_t[:, 0:128]
            ident = w_t[:, 128:256]

            def do_groups(a_t, h_ts, halves, groups, assign, racc, racc_col0):
                """Stationary-batched matmul pairs + abs-reduce for `groups` of
                one 128-row tile; group g -> racc[:, racc_col0 + g]."""
                for pi in range((len(groups) + 1) // 2):
                    pair = tuple(g for g in (2 * pi, 2 * pi + 1)
                                 if g < len(groups))
                    ps_l = []
                    for gi in pair:
                        ps = pspool.tile([128, 1024], f32, tag="ps")
                        ps_l.append(ps)
                    for k, gi in enumerate(pair):
                        gs, ge, chunks = groups[gi]
                        for (cs, ce) in chunks:
                            cw = ce - cs
                            o = cs - gs
                            nc.tensor.matmul(ps_l[k][:, o:o + cw], s4,
                                             a_t[:, cs:ce],
                                             start=True, stop=False)
                    for k, gi in enumerate(pair):
                        gs, ge, chunks = groups[gi]
                        hi = 0
                        while hi + 1 < len(halves) and gs >= halves[hi + 1][0]:
                            hi += 1
                        hb = halves[hi][0]
                        h_t = h_ts[hi]
                        for (cs, ce) in chunks:
                            cw = ce - cs
                            o = cs - gs
                            nc.tensor.matmul(ps_l[k][:, o:o + cw], ident,
                                             h_t[:, cs - hb:ce - hb],
                                             start=False, stop=True)
                    if mode != "full":
                        continue
                    for k, gi in enumerate(pair):
                        gs, ge, chunks = groups[gi]
                        gw = ge - gs
                        rc = racc_col0 + gi
                        if assign[gi] == "S":
                            sc = scpool.tile([128, 1024], f32, tag="sc")
                            nc.scalar.activation(
                                sc[:, :gw], ps_l[k][:, :gw],
                                mybir.ActivationFunctionType.Abs,
                                accum_out=racc[:, rc:rc + 1])
                        else:
                            nc.vector.tensor_reduce(
                                racc[:, rc:rc + 1], ps_l[k][:, :gw],
                                axis=mybir.AxisListType.X,
                                op=mybir.AluOpType.add,
                                apply_absolute_value=True)

            for _rep in range(reps):
                for ti in range(TILES_PER_CORE):
                    r0 = TILE_OUT * ti
                    a_t = apool.tile([128, N], bf16, tag="a")
                    nc.sync.dma_start(out=a_t[:, :], in_=a_dram[r0:r0 + 128, :])
                    if mode == "dma":
                        continue

                    # fused horizontal adds, one per half-tile
                    h_ts = []
                    for (hb, he) in _HALVES:
                        hw = he - hb
                        h_t = hpool.tile([128, 4096], bf16, tag="h")
                        nc.vector.tensor_add(h_t[:, :hw],
                                             a_t[:, hb - 1:hb - 1 + hw],
                                             a_t[:, hb + 1:hb + 1 + hw])
                        h_ts.append(h_t)
                    if mode == "dve":
                        continue

                    if mode == "full":
                        racc = rpool.tile([128, 8], f32, tag="racc")
                    else:
                        racc = None
                    do_groups(a_t, h_ts, _HALVES, _GROUPS, _ASSIGN, racc, 0)
                    if racc is not None:
                        nc.gpsimd.dma_start(
                            out=out_dram[TILE_OUT * ti:TILE_OUT * ti + TILE_OUT,
                                         0:8],
                            in_=racc[1:127, 0:8])

                # tail slab: [128, 1024], outputs slab cols 1..1022
                a2_t = apool.tile([128, N], bf16, tag="a")
                nc.sync.dma_start(out=a2_t[:, 0:TAIL_W], in_=a2_dram[:, :])
                # strip: [128, 16], outputs strip cols 1..14
                a3_t = apool.tile([128, N], bf16, tag="a")
                nc.sync.dma_start(out=a3_t[:, 0:STRIP_W], in_=a3_dram[:, :])
                if mode == "dma":
                    continue

                h2_t = hpool.tile([128, 4096], bf16, tag="h")
                nc.vector.tensor_add(h2_t[:, 0:TAIL_OUT],
                                     a2_t[:, 0:TAIL_OUT],
                                     a2_t[:, 2:2 + TAIL_OUT])
                h3_t = hpool.tile([128, 4096], bf16, tag="h")
                nc.vector.tensor_add(h3_t[:, 0:STRIP_W - 2],
                                     a3_t[:, 0:STRIP_W - 2],
                                     a3_t[:, 2:STRIP_W])
                if mode == "dve":
                    continue

                if mode == "full":
                    racc2 = rpool.tile([128, 8], f32, tag="racc")
                else:
                    racc2 = None
                # tail: one 1024-col group, chunks (1,513),(513,1023)
                tail_groups = [(1, 1023, [(1, 513), (513, 1023)])]
                do_groups(a2_t, [h2_t], [(1, 1023)], tail_groups, "S",
                          racc2, 0)
                strip_groups = [(1, 15, [(1, 15)])]
                do_groups(a3_t, [h3_t], [(1, 15)], strip_groups, "V",
                          racc2, 1)
                if racc2 is not None:
                    nc.gpsimd.dma_start(out=out_dram[0:128, 8:9],
                                        in_=racc2[:, 0:1])
                    nc.gpsimd.dma_start(out=out_dram[0:128, 9:10],
                                        in_=racc2[:, 1:2])
    nc.compile()
    return nc


def _get_nc(reps=1, mode="full"):
    key = (reps, mode)
    if key not in _NC_CACHE:
        _NC_CACHE[key] = _build_nc(reps, mode)
    return _NC_CACHE[key]


def _weight_matrix():
    import ml_dtypes
    w = np.zeros((256, 128), dtype=np.float32)
    idx = np.arange(128)
    w[idx, idx] = -4.0
    w[idx[:-1], idx[:-1] + 1] = 1.0
    w[idx[1:], idx[1:] - 1] = 1.0
    w[128 + idx, idx] = 1.0
    return w.astype(ml_dtypes.bfloat16)


def _make_inmaps(adj):
    """Per-core input dicts: main shard [1010, 8192], tail slab [128, 1024],
    strip [128, 16] (core 0 real, others zero), weights."""
    import ml_dtypes
    adj_bf = adj.astype(ml_dtypes.bfloat16)
    w = _weight_matrix()
    zstrip = np.zeros((128, STRIP_W), dtype=ml_dtypes.bfloat16)
    strip0 = np.ascontiguousarray(adj_bf[TAIL_ROW0:, STRIP_C0:STRIP_C0 + STRIP_W])
    maps = []
    for k in range(NCORES):
        r0 = k * TILES_PER_CORE * TILE_OUT
        shard = np.ascontiguousarray(adj_bf[r0:r0 + SHARD_ROWS])
        c0 = k * TAIL_OUT
        slab = np.ascontiguousarray(adj_bf[TAIL_ROW0:, c0:c0 + TAIL_W])
        maps.append({"a": shard, "a2": slab,
                     "a3": strip0 if k == 0 else zstrip, "w": w})
    return maps


def _host_reduce(results):
    total = 0.0
    for k in range(NCORES):
        out = np.asarray(results[k]["out"], dtype=np.float64)
        total += out[0:1008, 0:8].sum()
        total += out[1:127, 8].sum()
        total += out[1:127, 9].sum()
    return np.asarray(LAMBDA_SMOOTH * 0.25 * total, dtype=np.float32)


def kernel(adj: np.ndarray) -> np.ndarray:
    import time
    from concourse.bass_utils import run_bass_kernel_spmd

    adj = np.asarray(adj, dtype=np.float32)
    assert adj.shape == (N, N)

    nc = _get_nc()
    in_maps = _make_inmaps(adj)
    last_err = None
    for attempt in range(3):
        try:
            res = run_bass_kernel_spmd(nc, in_maps, list(range(NCORES)))
            return _host_reduce(res.results)
        except Exception as e:  # transient accelerator failures: back off, retry
            last_err = e
            time.sleep(45 * (attempt + 1))
    raise last_err


# revision 8
# speedup vs baseline: 1.0056x; 1.0056x over previous
"""Trainium2 Bass kernel: 5-point Laplacian smoothness loss over an 8192x8192
float32 matrix, sharded across 8 NeuronCores.

loss = 0.1 * sum_{i,j interior} | a[i,j] - 0.25*(a[i-1,j]+a[i+1,j]+a[i,j-1]+a[i,j+1]) |
     = 0.025 * sum | 4a[i,j] - (a[i-1,j]+a[i+1,j]+a[i,j-1]+a[i,j+1]) |

Per-core plan (PE-bound at ~216 ns per 512-col bf16 matmul):
  - host casts to bf16; global interior rows 1..8190 are covered by 65 row
    tiles of 126 output rows each (128 input rows, 2-row overlap). 64 tiles
    are distributed 8-per-core via contiguous [1010, 8192] shards; the 65th
    (rows 8065..8190) is split column-wise across the cores (1022 output
    cols each via a [128, 1024] slab), plus a shared 14-col strip computed
    only with core 0's data (other cores get a zero strip). This removes
    the baseline's 9th near-empty row tile (-11% PE work).
  - per 128-row tile: vertical stencil via TensorE matmul with tridiagonal
    (1,-4,1) bf16 stationary into PSUM; horizontal l+r via one fused DVE
    shifted bf16 add per half-tile, accumulated with identity matmuls.
    Matmuls are emitted in stationary-batched quads (T,T,T,T then I,I,I,I).
  - PSUM: 8 groups of 1024 f32 cols (2 banks each), 4 group-buffers in
    flight so the abs-reduce latency never stalls the PE.
  - abs+row-sum from PSUM: ScalarE activation(Abs, accum_out) on 6 groups,
    VectorE tensor_reduce(abs) on 2 (DVE also does the h-adds; measured
    rates: ScalarE (FD+352)/1.2ns, DVE h-add (58+FD/2)/0.96ns, DVE reduce
    (120+FD)/0.96ns -> ScalarE ~56.2us, DVE ~55.5us, PE ~56.2us; measured
    HW exec ~56.0us, i.e. at the PE floor of this 2-pass bf16 design).
    fp8 DoubleRow matmuls (0.5 cyc/col) would cut PE to ~34us but any
    DoubleRow matmul that is not the first matmul of the program wedges
    the device (NRT_EXEC_UNIT_UNRECOVERABLE), and walrus codegen rejects
    ldw-opt + SwInterleave variants, so that path is unusable here.
  - output [1024, 10] f32 per core: cols 0:8 per-row-group sums of the 8
    main tiles, col 8 the tail slab, col 9 the strip; host does the final
    float64 sum * 0.025.
"""

import numpy as np

N = 8192
NCORES = 8
TILE_OUT = 126                        # output rows per main tile
TILES_PER_CORE = 8
SHARD_ROWS = TILES_PER_CORE * TILE_OUT + 2   # 1010
TAIL_ROW0 = 64 * TILE_OUT                    # 8064: input row 0 of tail tile
TAIL_W = 1024                          # tail slab input cols per core
TAIL_OUT = 1022                        # valid output cols per tail slab
STRIP_C0 = 8 * TAIL_OUT                # 8176: strip input col 0
STRIP_W = 16                           # strip input cols (outputs 8177..8190)
LAMBDA_SMOOTH = 0.1
NG = 8                                 # PSUM groups per main tile (1024 cols)

# center-column chunks (512-wide, last 510); groups of 2 chunks
_CHUNKS = [(1 + 512 * i, min(1 + 512 * (i + 1), 8191)) for i in range(16)]
_GROUPS = [(_CHUNKS[2 * g][0], _CHUNKS[2 * g + 1][1], _CHUNKS[2 * g:2 * g + 2])
           for g in range(NG)]
# abs-reduce engine per group: 'S' = ScalarE activation, 'V' = DVE reduce.
# DVE's groups come first so its reduces drain early each tile, freeing the
# DVE queue for the next tile's h-adds before the PE needs them.
_ASSIGN = "VVSSSSSS"
# h halves: half 0 covers groups 0-3 (cols 1..4097), half 1 groups 4-7
_HALVES = [(1, 4097), (4097, 8191)]

_NC_CACHE = {}


def _build_nc(reps=1, mode="full"):
    """mode: 'full' | 'dma' (loads only) | 'dve' (+h-adds) | 'pe' (+matmuls)."""
    import concourse.tile as tile
    from concourse import bacc, mybir

    f32 = mybir.dt.float32
    bf16 = mybir.dt.bfloat16
    nc = bacc.Bacc("TRN2", target_bir_lowering=False, debug=False)
    a_dram = nc.declare_dram_parameter("a", [SHARD_ROWS, N], bf16, isOutput=False)
    a2_dram = nc.declare_dram_parameter("a2", [128, TAIL_W], bf16, isOutput=False)
    a3_dram = nc.declare_dram_parameter("a3", [128, STRIP_W], bf16, isOutput=False)
    w_dram = nc.declare_dram_parameter("w", [256, 128], bf16, isOutput=False)
    out_dram = nc.declare_dram_parameter("out", [1024, 10], f32, isOutput=True)

    with tile.TileContext(nc) as tc:
        with (
            tc.tile_pool(name="wpool", bufs=1) as wpool,
            tc.tile_pool(name="apool", bufs=5) as apool,
            tc.tile_pool(name="hpool", bufs=4) as hpool,
            tc.tile_pool(name="scpool", bufs=4) as scpool,
            tc.tile_pool(name="rpool", bufs=3) as rpool,
            tc.tile_pool(name="pspool", bufs=4, space="PSUM") as pspool,
        ):
            # stationary matrices: cols 0:128 tridiag(1,-4,1), cols 128:256 identity
            w_t = wpool.tile([128, 256], bf16)
            nc.gpsimd.dma_start(out=w_t[:, 0:128], in_=w_dram[0:128, :])
            nc.gpsimd.dma_start(out=w_t[:, 128:256], in_=w_dram[128:256, :])
            s4 = w_t[:, 0:128]
            ident = w_t[:, 128:256]

            def do_groups(a_t, h_ts, halves, groups, assign, racc, racc_col0):
                """Stationary-batched matmul pairs + abs-reduce for `groups` of
                one 128-row tile; group g -> racc[:, racc_col0 + g]."""
                for pi in range((len(groups) + 1) // 2):
                    pair = tuple(g for g in (2 * pi, 2 * pi + 1)
                                 if g < len(groups))
                    ps_l = []
                    for gi in pair:
                        ps = pspool.tile([128, 1024], f32, tag="ps")
                        ps_l.append(ps)
                    for k, gi in enumerate(pair):
                        gs, ge, chunks = groups[gi]
                        for (cs, ce) in chunks:
                            cw = ce - cs
                            o = cs - gs
                            nc.tensor.matmul(ps_l[k][:, o:o + cw], s4,
                                             a_t[:, cs:ce],
                                             start=True, stop=False)
                    for k, gi in enumerate(pair):
                        gs, ge, chunks = groups[gi]
                        hi = 0
                        while hi + 1 < len(halves) and gs >= halves[hi + 1][0]:
                            hi += 1
                        hb = halves[hi][0]
                        h_t = h_ts[hi]
                        for (cs, ce) in chunks:
                            cw = ce - cs
                            o = cs - gs
                            nc.tensor.matmul(ps_l[k][:, o:o + cw], ident,
                                             h_t[:, cs - hb:ce - hb],
                                             start=False, stop=True)
                    if mode != "full":
                        continue
                    for k, gi in enumerate(pair):
                        gs, ge, chunks = groups[gi]
                        gw = ge - gs
                        rc = racc_col0 + gi
                        if assign[gi] == "S":
                            sc = scpool.tile([128, 1024], f32, tag="sc")
                            nc.scalar.activation(
                                sc[:, :gw], ps_l[k][:, :gw],
                                mybir.ActivationFunctionType.Abs,
                                accum_out=racc[:, rc:rc + 1])
                        else:
                            nc.vector.tensor_reduce(
                                racc[:, rc:rc + 1], ps_l[k][:, :gw],
                                axis=mybir.AxisListType.X,
                                op=mybir.AluOpType.add,
                                apply_absolute_value=True)

            for _rep in range(reps):
                for ti in range(TILES_PER_CORE):
                    r0 = TILE_OUT * ti
                    a_t = apool.tile([128, N], bf16, tag="a")
                    nc.sync.dma_start(out=a_t[:, :], in_=a_dram[r0:r0 + 128, :])
                    if mode == "dma":
                        continue

                    # fused horizontal adds, one per half-tile
                    h_ts = []
                    for (hb, he) in _HALVES:
                        hw = he - hb
                        h_t = hpool.tile([128, 4096], bf16, tag="h")
                        nc.vector.tensor_add(h_t[:, :hw],
                                             a_t[:, hb - 1:hb - 1 + hw],
                                             a_t[:, hb + 1:hb + 1 + hw])
                        h_ts.append(h_t)
                    if mode == "dve":
                        continue

                    if mode == "full":
                        racc = rpool.tile([128, 8], f32, tag="racc")
                    else:
                        racc = None
                    do_groups(a_t, h_ts, _HALVES, _GROUPS, _ASSIGN, racc, 0)
                    if racc is not None:
                        nc.gpsimd.dma_start(
                            out=out_dram[TILE_OUT * ti:TILE_OUT * ti + TILE_OUT,
                                         0:8],
                            in_=racc[1:127, 0:8])

                # tail slab: [128, 1024], outputs slab cols 1..1022
                a2_t = apool.tile([128, N], bf16, tag="a")
                nc.sync.dma_start(out=a2_t[:, 0:TAIL_W], in_=a2_dram[:, :])
                # strip: [128, 16], outputs strip cols 1..14
                a3_t = apool.tile([128, N], bf16, tag="a")
                nc.sync.dma_start(out=a3_t[:, 0:STRIP_W], in_=a3_dram[:, :])
                if mode == "dma":
                    continue

                h2_t = hpool.tile([128, 4096], bf16, tag="h")
                nc.vector.tensor_add(h2_t[:, 0:TAIL_OUT],
                                     a2_t[:, 0:TAIL_OUT],
                                     a2_t[:, 2:2 + TAIL_OUT])
                h3_t = hpool.tile([128, 4096], bf16, tag="h")
                nc.vector.tensor_add(h3_t[:, 0:STRIP_W - 2],
                                     a3_t[:, 0:STRIP_W - 2],
                                     a3_t[:, 2:STRIP_W])
                if mode == "dve":
                    continue

                if mode == "full":
                    racc2 = rpool.tile([128, 8], f32, tag="racc")
                else:
                    racc2 = None
                # tail: one 1024-col group, chunks (1,513),(513,1023)
                tail_groups = [(1, 1023, [(1, 513), (513, 1023)])]
                do_groups(a2_t, [h2_t], [(1, 1023)], tail_groups, "S",
                          racc2, 0)
                strip_groups = [(1, 15, [(1, 15)])]
                do_groups(a3_t, [h3_t], [(1, 15)], strip_groups, "V",
                          racc2, 1)
                if racc2 is not None:
                    nc.gpsimd.dma_start(out=out_dram[0:128, 8:9],
                                        in_=racc2[:, 0:1])
                    nc.gpsimd.dma_start(out=out_dram[0:128, 9:10],
                                        in_=racc2[:, 1:2])
    nc.compile()
    return nc


def _get_nc(reps=1, mode="full"):
    key = (reps, mode)
    if key not in _NC_CACHE:
        _NC_CACHE[key] = _build_nc(reps, mode)
    return _NC_CACHE[key]


def _weight_matrix():
    import ml_dtypes
    w = np.zeros((256, 128), dtype=np.float32)
    idx = np.arange(128)
    w[idx, idx] = -4.0
    w[idx[:-1], idx[:-1] + 1] = 1.0
    w[idx[1:], idx[1:] - 1] = 1.0
    w[128 + idx, idx] = 1.0
    return w.astype(ml_dtypes.bfloat16)


def _make_inmaps(adj):
    """Per-core input dicts: main shard [1010, 8192], tail slab [128, 1024],
    strip [128, 16] (core 0 real, others zero), weights."""
    import ml_dtypes
    adj_bf = adj.astype(ml_dtypes.bfloat16)
    w = _weight_matrix()
    zstrip = np.zeros((128, STRIP_W), dtype=ml_dtypes.bfloat16)
    strip0 = np.ascontiguousarray(adj_bf[TAIL_ROW0:, STRIP_C0:STRIP_C0 + STRIP_W])
    maps = []
    for k in range(NCORES):
        r0 = k * TILES_PER_CORE * TILE_OUT
        shard = np.ascontiguousarray(adj_bf[r0:r0 + SHARD_ROWS])
        c0 = k * TAIL_OUT
        slab = np.ascontiguousarray(adj_bf[TAIL_ROW0:, c0:c0 + TAIL_W])
        maps.append({"a": shard, "a2": slab,
                     "a3": strip0 if k == 0 else zstrip, "w": w})
    return maps


def _host_reduce(results):
    total = 0.0
    for k in range(NCORES):
        out = np.asarray(results[k]["out"], dtype=np.float64)
        total += out[0:1008, 0:8].sum()
        total += out[1:127, 8].sum()
        total += out[1:127, 9].sum()
    return np.asarray(LAMBDA_SMOOTH * 0.25 * total, dtype=np.float32)


def kernel(adj: np.ndarray) -> np.ndarray:
    import time
    from concourse.bass_utils import run_bass_kernel_spmd

    adj = np.asarray(adj, dtype=np.float32)
    assert adj.shape == (N, N)

    nc = _get_nc()
    in_maps = _make_inmaps(adj)
    last_err = None
    for attempt in range(3):
        try:
            res = run_bass_kernel_spmd(nc, in_maps, list(range(NCORES)))
            return _host_reduce(res.results)
        except Exception as e:  # transient accelerator failures: back off, retry
            last_err = e
            time.sleep(45 * (attempt + 1))
    raise last_err
